# revision 20
# baseline (speedup 1.0000x reference)
"""Trainium2 Bass kernel for nn_Bottleneck_75325136437765 (sparse 3x3 local attention bottleneck).

Sharding: data-parallel over batch B=16 across 8 cores (2 batches/core), params replicated.

Per-core layout: channels on partitions, spatial (32*32=1024) on free dim. All matmuls bf16
(fp32 matmuls cost 2 PE passes on TRN2), fp32 PSUM accumulation everywhere.

The WIDTH=256 channel space is PERMUTED to g-major d-minor layout:
    partition chunk pt = d//4,  local partition p = g*4 + (d%4)   (c = g*8+d, 32 heads, d<8)
so every head->channel broadcast (softmax numerator e and 1/den) is the same
[[stride,32],[0,4]] partition AP (each of 32 head rows replicated into 4
consecutive partitions) and is shared by BOTH chunks: 10 broadcast DMAs per
batch instead of 20, spread across the sync/scalar/gpsimd DMA queues.

The two batches are software-pipelined (instruction emission interleaved) so the PE queue
always has independent matmul work during the other batch's DVE/DMA-bound attention
stages, which also keeps the PE HAM clock-gate warm (2.4GHz). A few warm-up matmuls on a
zeroed tile run during the initial x/w DMA so conv1 starts at full clock.

  conv1/qkv/conv3: plain matmuls (lhsT = transposed weights, host-precomputed, bn scales folded).
  attention logits, packed PSUM layout (row = 32*(kk%4) + head, 3 tiles of 4 shifts):
      L[g,kk,hw] = sum_d q[gd,hw]*k[gd,hw+off_kk]  (col-tiled 0/1-selection matmuls over products)
                 + sum_d q[gd,hw]*pos[gd,kk]       (P2 matmul, accumulated into same PSUM)
  softmax over kk without max-subtraction, 1/sum factored out to the end:
      e = exp(L) (packed, 3 ACT ops); den = sum_kk e via 0/1 matmuls; recip = 1/den
      out_pre[c,hw] = sum_kk e_bc[c,kk,hw] * v[c,hw+off_kk]
        e_bc via rep4 broadcast DMA; per-shift product on DVE;
        sum over kk via identity-matmul PSUM accumulation
      h2 = relu(out_pre * recip_bc + bnatt_b)
  residual: bf16 x re-streamed through identity matmul into the conv3 PSUM.
  output stored bf16 (within tolerance), host converts to fp32.
"""

import numpy as np

import concourse.bass as bass
import concourse.bacc as bacc
import concourse.tile as tile
from concourse import mybir
from concourse.bass_utils import run_bass_kernel_spmd

# ---- problem constants (hardcoded per contract) ----
B, CIN, H, W = 16, 1024, 32, 32
WIDTH, OUT, HEADS, KS = 256, 1024, 32, 3
D = WIDTH // HEADS            # 8 channels per head
HW = H * W                    # 1024
NC_ = 8                       # cores
BL = B // NC_                 # 2 batches per core
P = 128
KC1 = CIN // P                # 8 contraction chunks for conv1
PT = WIDTH // P               # 2 partition tiles for width-256 tensors
OC = OUT // P                 # 8 output ptiles for conv3
NKK = KS * KS                 # 9 shifts
NT = 3                        # packed logit tiles (4+4+1 shifts)
F32 = mybir.dt.float32
BF16 = mybir.dt.bfloat16
NHALF = 2                     # PSUM-bank limit: matmul N<=512 fp32 out


def _ns(n):
    return slice(n * 512, (n + 1) * 512)


def build_program():
    nc = bacc.Bacc(None, target_bir_lowering=False, debug=False)

    def din(name, shape, dt=BF16):
        return nc.dram_tensor(name, list(shape), dt, kind="ExternalInput").ap()

    x16_d = din("x16", (BL, KC1, P, HW))
    w1T_d = din("w1T", (KC1, P, WIDTH))
    wqT_d = din("wqT", (PT, P, WIDTH))
    wkT_d = din("wkT", (PT, P, WIDTH))
    wvT_d = din("wvT", (PT, P, WIDTH))
    w3T_d = din("w3T", (PT, P, OUT))
    b1_d = din("b1", (PT, P, 1), F32)
    bq_d = din("bq", (PT, P, 1), F32)
    bk_d = din("bk", (PT, P, 1), F32)
    bv_d = din("bv", (PT, P, 1), F32)
    batt_d = din("batt", (PT, P, 1), F32)
    b3_d = din("b3", (OC, P, 1), F32)
    sel_d = din("sel", (P, HEADS))
    p2_d = din("p2", (PT, P, NT, P))
    sab_d = din("sab", (P, HEADS))
    eye32_d = din("eye32", (HEADS, HEADS))
    ident_d = din("ident", (P, P))
    out_d = nc.dram_tensor("out", [BL, OC, P, HW], BF16, kind="ExternalOutput").ap()

    with tile.TileContext(nc) as tc:
        with (
            tc.tile_pool(name="consts", bufs=1) as consts,
            tc.tile_pool(name="xb", bufs=2) as xbp,
            tc.tile_pool(name="act", bufs=2) as actp,
            tc.tile_pool(name="attn", bufs=2) as attnp,
            tc.tile_pool(name="tmp", bufs=5) as tmpp,
            tc.tile_pool(name="tmp2", bufs=4) as tmp2p,
            tc.tile_pool(name="ebc", bufs=9) as ebcp,
            tc.tile_pool(name="outz", bufs=3) as outzp,
            tc.tile_pool(name="pmm", bufs=2, space="PSUM") as pmm,
            tc.tile_pool(name="pL", bufs=1, space="PSUM") as pLp,
        ):
            # ---- warm-up: keep PE busy during initial DMA so HAM un-throttles ----
            warm = consts.tile([P, 512], BF16, tag="warm")
            nc.vector.memset(warm, 0.0)
            for _ in range(8):
                ps = pmm.tile([P, HW], F32, tag="mm")
                nc.tensor.matmul(ps[:, :512], warm[:, :P], warm,
                                 start=True, stop=True)

            # ---- load constants ----
            # constants other than w1T/b1 go on the SWDGE queue so the sync
            # queue serves conv1's x/w chunks first (fast kernel start)
            def cload(name, dram, shape, dt=BF16, re="k p m -> p k m"):
                t = consts.tile(shape, dt, tag=name)
                nc.gpsimd.dma_start(out=t, in_=dram.rearrange(re) if re else dram)
                return t

            w1T = consts.tile([P, KC1, WIDTH], BF16, tag="w1T")
            b1 = consts.tile([P, PT, 1], F32, tag="b1")
            nc.gpsimd.dma_start(out=b1, in_=b1_d.rearrange("k p m -> p k m"))
            wqT = cload("wqT", wqT_d, [P, PT, WIDTH])
            wkT = cload("wkT", wkT_d, [P, PT, WIDTH])
            wvT = cload("wvT", wvT_d, [P, PT, WIDTH])
            w3T = cload("w3T", w3T_d, [P, PT, OUT])
            bq = cload("bq", bq_d, [P, PT, 1], F32)
            bk = cload("bk", bk_d, [P, PT, 1], F32)
            bv = cload("bv", bv_d, [P, PT, 1], F32)
            batt = cload("batt", batt_d, [P, PT, 1], F32)
            b3 = cload("b3", b3_d, [P, OC, 1], F32)
            sel = cload("sel", sel_d, [P, HEADS], re=None)
            p2 = cload("p2", p2_d, [P, PT, NT, P], re="k p m o -> p k m o")
            sab = cload("sab", sab_d, [P, HEADS], re=None)
            eye32 = cload("eye32", eye32_d, [HEADS, HEADS], re=None)
            ident = cload("ident", ident_d, [P, P], re=None)

            def rep4_bcast(q_eng, dst, src32):
                # dst[p, :] = src32[p // 4, :] — each row into 4 consecutive partitions
                bc = bass.AP(tensor=src32.tensor, offset=src32.offset,
                             ap=[list(src32.ap[0]), [0, 4]]
                                + [list(a) for a in src32.ap[1:]])
                q_eng.dma_start(out=dst, in_=bc)

            # persistent zero-padded k tiles, double-buffered per batch
            # (borders stay zero: only the interior is ever written)
            kpads = []
            for i in range(2):
                kp_ = consts.tile([P, PT, H + 2, W + 2], BF16, tag=f"kpad{i}")
                nc.vector.memset(kp_, 0.0)
                kpads.append(kp_)

            # v-side shifted-accumulation buffers: the spatial shift of each
            # of the 9 taps is applied as a free SOURCE OFFSET in the e
            # broadcast DMA and as a free RHS OFFSET in the accumulation
            # matmul, so the e*v product itself is a fully contiguous
            # unit-stride DVE op (2x bf16 mode). PAD=34 zero elements flank
            # the 1024-wide interiors so shifted reads stay in-bounds.
            PAD = 34
            WPAD = PAD + HW + PAD
            epk2s = []
            for i in range(2 * NT):
                ep_ = consts.tile([P, WPAD], BF16, tag=f"epk2_{i}")
                nc.vector.memset(ep_[:, :PAD], 0.0)
                nc.vector.memset(ep_[:, PAD + HW:], 0.0)
                epk2s.append(ep_)
            t2ps = []
            for i in range(3):
                t2_ = consts.tile([P, PT, WPAD], BF16, tag=f"t2p{i}")
                nc.vector.memset(t2_[:, :, :PAD], 0.0)
                nc.vector.memset(t2_[:, :, PAD + HW:], 0.0)
                t2ps.append(t2_)

            # per-batch state
            st = [dict() for _ in range(BL)]

            def load_x(b):
                xb = xbp.tile([P, KC1, HW], BF16, tag="xb")
                st[b]["xb"] = xb
                for kc in range(KC1):
                    if b == 0:
                        # separate HWDGE queue so w1T and x stream in parallel
                        nc.scalar.dma_start(out=w1T[:, kc, :], in_=w1T_d[kc])
                    nc.sync.dma_start(out=xb[:, kc, :], in_=x16_d[b, kc])

            def conv1(b):
                xb = st[b]["xb"]
                h1 = actp.tile([P, PT, HW], BF16, tag="h1")
                st[b]["h1"] = h1
                for mc in range(PT):
                    ps = pmm.tile([P, HW], F32, tag="mm")
                    for kc in range(KC1):
                        for n in range(NHALF):
                            nc.tensor.matmul(
                                ps[:, _ns(n)],
                                w1T[:, kc, mc * P:(mc + 1) * P],
                                xb[:, kc, _ns(n)],
                                start=(kc == 0), stop=(kc == KC1 - 1),
                            )
                    nc.scalar.activation(
                        out=h1[:, mc, :], in_=ps,
                        func=mybir.ActivationFunctionType.Relu,
                        bias=b1[:, mc], scale=1.0,
                    )

            def qkv_one(b, which):
                h1 = st[b]["h1"]
                kpad = kpads[b % 2]
                st[b]["kpad"] = kpad
                if which == "q":
                    q = actp.tile([P, PT, HW], BF16, tag="q")
                    st[b]["q"] = q
                elif which == "v":
                    v = actp.tile([P, PT, HW], BF16, tag="v")
                    st[b]["v"] = v
                wT, bias, relu = {
                    "q": (wqT, bq, True),
                    "k": (wkT, bk, True),
                    "v": (wvT, bv, False),
                }[which]
                for mc in range(PT):
                    ps = pmm.tile([P, HW], F32, tag="mm")
                    for kc in range(PT):
                        for n in range(NHALF):
                            nc.tensor.matmul(
                                ps[:, _ns(n)],
                                wT[:, kc, mc * P:(mc + 1) * P],
                                h1[:, kc, _ns(n)],
                                start=(kc == 0), stop=(kc == PT - 1),
                            )
                    if which == "k":
                        o = kpad[:, mc, 1:H + 1, 1:W + 1]
                        i = ps.rearrange("p (a b) -> p a b", a=H)
                    else:
                        o, i = st[b][which][:, mc, :], ps[:]
                    nc.scalar.activation(
                        out=o, in_=i,
                        func=(mybir.ActivationFunctionType.Relu if relu
                              else mybir.ActivationFunctionType.Identity),
                        bias=bias[:, mc], scale=1.0,
                    )

            def qkv(b):
                for which in ("q", "k", "v"):
                    qkv_one(b, which)

            def logits(b, filler=None):
                # packed tile t rows: 32*(kk%4) + g  for kk in {4t..4t+3}
                # filler: callbacks emitting independent PE work between
                # t-tiles so the PE queue never starves on DVE products
                q, kpad = st[b]["q"], st[b]["kpad"]
                epks = []
                st[b]["epks"] = epks
                den = attnp.tile([HEADS, HW], F32, tag="den")
                st[b]["den"] = den
                denp = pmm.tile([HEADS, HW], F32, tag="denp", bufs=1)
                for t in range(NT):
                    nsh = 4 if t < 2 else 1
                    rows = 32 * nsh
                    Lpk = pLp.tile([P, HW], F32, tag="Lpk")
                    st[b][f"Lpk{t}"] = Lpk
                    # qpos term: all rows at once per pt chunk
                    for n in range(NHALF):
                        for pt in range(PT):
                            nc.tensor.matmul(
                                Lpk[:rows, _ns(n)],
                                p2[:, pt, t, :rows],
                                q[:, pt, _ns(n)],
                                start=(pt == 0), stop=False,
                                skip_group_check=True,
                            )
                    # qk products (both chunks in one DVE op) + group reduce
                    for j in range(nsh):
                        kk = 4 * t + j
                        di, dj = kk // KS, kk % KS
                        tmp = tmpp.tile([P, PT, HW], BF16, tag="tmp")
                        nc.vector.tensor_tensor(
                            out=tmp.rearrange("p c (a b) -> p c a b", a=H),
                            in0=kpad[:, :, di:di + H, dj:dj + W],
                            in1=q.rearrange("p c (a b) -> p c a b", a=H),
                            op=mybir.AluOpType.mult,
                        )
                        for pt in range(PT):
                            for n in range(NHALF):
                                nc.tensor.matmul(
                                    Lpk[32 * j:32 * (j + 1), _ns(n)],
                                    sel,
                                    tmp[:, pt, _ns(n)],
                                    start=False, stop=(pt == PT - 1),
                                    tile_position=(0, 32 * j),
                                    skip_group_check=True,
                                )
                    epk = epk2s[NT * (b % 2) + t]
                    nc.scalar.activation(
                        out=epk[:rows, PAD:PAD + HW], in_=Lpk[:rows, :],
                        func=mybir.ActivationFunctionType.Exp,
                    )
                    epks.append(epk)
                    if filler is not None and t < len(filler):
                        filler[t]()
                # denominator accumulation (emitted after all exps so the PE
                # never waits on the ACT exp mid-phase)
                for t in range(NT):
                    rows = 128 if t < 2 else 32
                    lhs = sab if t < 2 else eye32
                    for n in range(NHALF):
                        nc.tensor.matmul(
                            denp[:, _ns(n)], lhs[:rows, :],
                            epks[t][:rows, PAD + n * 512:PAD + (n + 1) * 512],
                            start=(t == 0), stop=(t == NT - 1),
                            skip_group_check=True,
                        )
                nc.vector.reciprocal_approx_fast(out=den, in_=denp)

            def vphase(b):
                den, epks, v = st[b]["den"], st[b]["epks"], st[b]["v"]
                # recip broadcast head -> channels (same tile serves both chunks)
                recip_bc = attnp.tile([P, HW], F32, tag="recip_bc")
                rep4_bcast(nc.scalar, recip_bc, den)
                # e broadcasts, one per shift (shared by both chunks), with the
                # tap's spatial shift folded into the DMA source offset
                ebs = []
                for kk in range(NKK):
                    t, j = kk // 4, kk % 4
                    di, dj = kk // KS, kk % KS
                    dlt = 32 * (di - 1) + (dj - 1)
                    eb = ebcp.tile([P, HW], BF16, tag="ebc")
                    rep4_bcast(nc.scalar if kk % 2 == 0 else nc.gpsimd,
                               eb,
                               epks[t][32 * j:32 * (j + 1),
                                       PAD - dlt:PAD - dlt + HW])
                    # zero the column whose flat-shift read wrapped a row: the
                    # true neighbor there is off-grid (zero contribution)
                    if dj == 0:
                        nc.vector.memset(
                            eb.rearrange("p (a b) -> p a b", a=H)[:, :, W - 1],
                            0.0)
                    elif dj == 2:
                        nc.vector.memset(
                            eb.rearrange("p (a b) -> p a b", a=H)[:, :, 0],
                            0.0)
                    ebs.append(eb)
                h2 = actp.tile([P, PT, HW], BF16, tag="h2")
                st[b]["h2"] = h2
                accs = [pmm.tile([P, HW], F32, tag="mm", name=f"acc{i}")
                        for i in range(PT)]
                for kk in range(NKK):
                    di, dj = kk // KS, kk % KS
                    dlt = 32 * (di - 1) + (dj - 1)
                    # fully contiguous product for both chunks (2x bf16 DVE):
                    # eb broadcast along the chunk dim via 0-stride
                    ebbc = bass.AP(
                        tensor=ebs[kk].tensor, offset=ebs[kk].offset,
                        ap=[list(ebs[kk].ap[0]), [0, PT], [1, HW]])
                    t2 = t2ps[kk % 3]
                    nc.vector.tensor_tensor(
                        out=t2[:, :, PAD:PAD + HW],
                        in0=ebbc,
                        in1=v,
                        op=mybir.AluOpType.mult,
                    )
                    for mc in range(PT):
                        for n in range(NHALF):
                            nc.tensor.matmul(
                                accs[mc][:, _ns(n)], ident,
                                t2[:, mc,
                                   PAD + dlt + n * 512:PAD + dlt + (n + 1) * 512],
                                start=(kk == 0), stop=(kk == NKK - 1),
                                skip_group_check=True,
                            )
                for mc in range(PT):
                    # h2 = relu(acc * recip_bc + batt)
                    t3 = tmp2p.tile([P, HW], F32, tag="t3", bufs=2)
                    nc.vector.tensor_tensor(
                        out=t3, in0=accs[mc], in1=recip_bc,
                        op=mybir.AluOpType.mult,
                    )
                    nc.scalar.activation(
                        out=h2[:, mc, :], in_=t3,
                        func=mybir.ActivationFunctionType.Relu,
                        bias=batt[:, mc], scale=1.0,
                    )

            def conv3_chunk(b, ocs, res_dve=False):
                # res_dve: add the residual on DVE instead of a PE identity
                # matmul — used for the last batch where the PE is the tail
                # pacer and DVE/ACT are otherwise idle
                xb, h2 = st[b]["xb"], st[b]["h2"]
                for oc in ocs:
                    ps = pmm.tile([P, HW], F32, tag="mm")
                    for n in range(NHALF):
                        for kc in range(PT):
                            nc.tensor.matmul(
                                ps[:, _ns(n)],
                                w3T[:, kc, oc * P:(oc + 1) * P],
                                h2[:, kc, _ns(n)],
                                start=(kc == 0),
                                stop=(res_dve and kc == PT - 1),
                                skip_group_check=True,
                            )
                        if not res_dve:
                            nc.tensor.matmul(
                                ps[:, _ns(n)], ident, xb[:, oc, _ns(n)],
                                start=False, stop=True,
                                skip_group_check=True,
                            )
                    zr = outzp.tile([P, HW], BF16, tag="outzr")
                    if res_dve:
                        tr = tmp2p.tile([P, HW], F32, tag="tres", bufs=2)
                        nc.vector.tensor_tensor(
                            out=tr, in0=ps, in1=xb[:, oc, :],
                            op=mybir.AluOpType.add,
                        )
                        src = tr
                    else:
                        src = ps
                    nc.scalar.activation(
                        out=zr, in_=src, func=mybir.ActivationFunctionType.Relu,
                        bias=b3[:, oc], scale=1.0,
                    )
                    (nc.sync if oc % 2 == 0 else nc.scalar).dma_start(
                        out=out_d[b, oc], in_=zr)

            # ---- software-pipelined emission: PE always has independent
            # matmul work queued while the other batch's attention runs ----
            load_x(0)
            load_x(1)
            conv1(0)
            conv1(1)
            qkv(0)
            logits(0, filler=[lambda: qkv_one(1, "q"),
                              lambda: qkv_one(1, "k"),
                              lambda: qkv_one(1, "v")])
            vphase(0)
            logits(1, filler=[lambda: conv3_chunk(0, (0, 1, 2)),
                              lambda: conv3_chunk(0, (3, 4, 5)),
                              lambda: conv3_chunk(0, (6, 7))])
            vphase(1)
            conv3_chunk(1, range(OC), res_dve=True)

    nc.compile()
    return nc


_PROG = None


def _perm():
    # channel permutation: c=g*8+d -> pt=d//4, p=g*4+(d%4)
    pm = np.zeros(WIDTH, np.int64)
    for c in range(WIDTH):
        g, d = c // D, c % D
        pm[(d // 4) * P + g * 4 + (d % 4)] = c
    return pm


def _host_prep(inputs):
    import ml_dtypes
    bf = ml_dtypes.bfloat16
    f = lambda a: np.asarray(a, dtype=np.float32)
    x = f(inputs["x"])
    pm = _perm()
    # fold bn scales into weights (bn(conv(x,W),s,b) = conv(x, s*W) + b)
    w1 = f(inputs["w_conv1"]) * f(inputs["bn1_s"])[:, None]
    wq = (f(inputs["wq"]) * f(inputs["bnq_s"])[:, None])[pm]
    wk = (f(inputs["wk"]) * f(inputs["bnk_s"])[:, None])[pm]
    # fold bnatt scale through the (linear) attention-value path into v
    sv = f(inputs["bnatt_s"]) * f(inputs["bnv_s"])
    wv = (f(inputs["wv"]) * sv[:, None])[pm]
    bv = (f(inputs["bnatt_s"]) * f(inputs["bnv_b"]))[pm]
    bq = f(inputs["bnq_b"])[pm]
    bk = f(inputs["bnk_b"])[pm]
    batt = f(inputs["bnatt_b"])[pm]
    w3 = (f(inputs["w_conv3"]) * f(inputs["bn3_s"])[:, None])[:, pm]

    posf = (f(inputs["pos_h"]) + f(inputs["pos_w"])).reshape(WIDTH, NKK)

    # sel[p, g] = 1 iff p // 4 == g  (same for both chunks in permuted layout)
    sel = np.zeros((P, HEADS), np.float32)
    for p in range(P):
        sel[p, p // 4] = 1.0
    # p2[pt, p, t, 32*j+g] = pos[c_global, 4t+j] if g == p // 4
    p2 = np.zeros((PT, P, NT, P), np.float32)
    for pt in range(PT):
        for p in range(P):
            g = p // 4
            c = pm[pt * P + p]
            for kk in range(NKK):
                t, j = kk // 4, kk % 4
                p2[pt, p, t, 32 * j + g] = posf[c, kk]
    # sab[r, g] = 1 if r % 32 == g (sum over the 4 packed kk rows)
    sab = np.zeros((P, HEADS), np.float32)
    for r in range(P):
        sab[r, r % HEADS] = 1.0
    com = {
        "w1T": np.ascontiguousarray(w1.T.reshape(KC1, P, WIDTH)).astype(bf),
        "wqT": np.ascontiguousarray(wq.T.reshape(PT, P, WIDTH)).astype(bf),
        "wkT": np.ascontiguousarray(wk.T.reshape(PT, P, WIDTH)).astype(bf),
        "wvT": np.ascontiguousarray(wv.T.reshape(PT, P, WIDTH)).astype(bf),
        "w3T": np.ascontiguousarray(w3.T.reshape(PT, P, OUT)).astype(bf),
        "b1": f(inputs["bn1_b"]).reshape(PT, P, 1),
        "bq": bq.reshape(PT, P, 1),
        "bk": bk.reshape(PT, P, 1),
        "bv": bv.reshape(PT, P, 1),
        "batt": batt.reshape(PT, P, 1),
        "b3": f(inputs["bn3_b"]).reshape(OC, P, 1),
        "sel": sel.astype(bf),
        "p2": p2.astype(bf),
        "sab": sab.astype(bf),
        "eye32": np.eye(HEADS, dtype=np.float32).astype(bf),
        "ident": np.eye(P, dtype=np.float32).astype(bf),
    }
    xr = x.reshape(B, KC1, P, HW)
    in_maps = []
    for c in range(NC_):
        xs = np.ascontiguousarray(xr[c * BL:(c + 1) * BL])
        in_maps.append(dict(com, x16=xs.astype(bf)))
    return in_maps


def kernel(**inputs):
    global _PROG
    if _PROG is None:
        _PROG = build_program()
    in_maps = _host_prep(inputs)
    res = run_bass_kernel_spmd(_PROG, in_maps, core_ids=list(range(NC_)))
    outs = [np.asarray(res.results[c]["out"], dtype=np.float32)
            .reshape(BL, OUT, H, W) for c in range(NC_)]
    return np.concatenate(outs, axis=0)


# revision 24
# speedup vs baseline: 1.0934x; 1.0934x over previous
"""Trainium2 Bass kernel for nn_Bottleneck_75325136437765 (sparse 3x3 local attention bottleneck).

Sharding: data-parallel over batch B=16 across 8 cores (2 batches/core), params replicated.

Per-core layout: channels on partitions, spatial (32*32=1024) on free dim. All matmuls bf16
(fp32 matmuls cost 2 PE passes on TRN2), fp32 PSUM accumulation everywhere.

The WIDTH=256 channel space is PERMUTED to g-major d-minor layout:
    partition chunk pt = d//4,  local partition p = g*4 + (d%4)   (c = g*8+d, 32 heads, d<8)
so every head->channel broadcast (softmax numerator e and 1/den) is the same
[[stride,32],[0,4]] partition AP (each of 32 head rows replicated into 4
consecutive partitions) and is shared by BOTH chunks: 10 broadcast DMAs per
batch instead of 20, spread across the sync/scalar/gpsimd DMA queues.

The two batches are software-pipelined (instruction emission interleaved) so the PE queue
always has independent matmul work during the other batch's DVE/DMA-bound attention
stages, which also keeps the PE HAM clock-gate warm (2.4GHz). A few warm-up matmuls on a
zeroed tile run during the initial x/w DMA so conv1 starts at full clock.

  conv1/qkv/conv3: plain matmuls (lhsT = transposed weights, host-precomputed, bn scales folded).
  attention logits, packed PSUM layout (row = 32*(kk%4) + head, 3 tiles of 4 shifts):
      L[g,kk,hw] = sum_d q[gd,hw]*k[gd,hw+off_kk]  (col-tiled 0/1-selection matmuls over products)
                 + sum_d q[gd,hw]*pos[gd,kk]       (P2 matmul, accumulated into same PSUM)
  softmax over kk without max-subtraction, 1/sum factored out to the end:
      e = exp(L) (packed, 3 ACT ops); den = sum_kk e via 0/1 matmuls; recip = 1/den
      out_pre[c,hw] = sum_kk e_bc[c,kk,hw] * v[c,hw+off_kk]
        e_bc via rep4 broadcast DMA; per-shift product on DVE;
        sum over kk via identity-matmul PSUM accumulation
      h2 = relu(out_pre * recip_bc + bnatt_b)
  residual: bf16 x re-streamed through identity matmul into the conv3 PSUM.
  output stored bf16 (within tolerance), host converts to fp32.
"""

import numpy as np

import concourse.bass as bass
import concourse.bacc as bacc
import concourse.tile as tile
from concourse import mybir
from concourse.bass_utils import run_bass_kernel_spmd

# ---- problem constants (hardcoded per contract) ----
B, CIN, H, W = 16, 1024, 32, 32
WIDTH, OUT, HEADS, KS = 256, 1024, 32, 3
D = WIDTH // HEADS            # 8 channels per head
HW = H * W                    # 1024
NC_ = 8                       # cores
BL = B // NC_                 # 2 batches per core
P = 128
KC1 = CIN // P                # 8 contraction chunks for conv1
PT = WIDTH // P               # 2 partition tiles for width-256 tensors
OC = OUT // P                 # 8 output ptiles for conv3
NKK = KS * KS                 # 9 shifts
NT = 3                        # packed logit tiles (4+4+1 shifts)
F32 = mybir.dt.float32
BF16 = mybir.dt.bfloat16
NHALF = 2                     # PSUM-bank limit: matmul N<=512 fp32 out


def _ns(n):
    return slice(n * 512, (n + 1) * 512)


def build_program():
    nc = bacc.Bacc(None, target_bir_lowering=False, debug=False)

    def din(name, shape, dt=BF16):
        return nc.dram_tensor(name, list(shape), dt, kind="ExternalInput").ap()

    x16_d = din("x16", (BL, KC1, P, HW))
    w1T_d = din("w1T", (KC1, P, WIDTH))
    wqT_d = din("wqT", (PT, P, WIDTH))
    wkT_d = din("wkT", (PT, P, WIDTH))
    wvT_d = din("wvT", (PT, P, WIDTH))
    w3T_d = din("w3T", (PT, P, OUT))
    b1_d = din("b1", (PT, P, 1), F32)
    bq_d = din("bq", (PT, P, 1), F32)
    bk_d = din("bk", (PT, P, 1), F32)
    bv_d = din("bv", (PT, P, 1), F32)
    batt_d = din("batt", (PT, P, 1), F32)
    b3_d = din("b3", (OC, P, 1), F32)
    sel_d = din("sel", (P, HEADS))
    p2_d = din("p2", (PT, P, NT, P))
    sab_d = din("sab", (P, HEADS))
    eye32_d = din("eye32", (HEADS, HEADS))
    ident_d = din("ident", (P, P))
    out_d = nc.dram_tensor("out", [BL, OC, P, HW], BF16, kind="ExternalOutput").ap()

    with tile.TileContext(nc) as tc:
        with (
            tc.tile_pool(name="consts", bufs=1) as consts,
            tc.tile_pool(name="xb", bufs=2) as xbp,
            tc.tile_pool(name="act", bufs=2) as actp,
            tc.tile_pool(name="attn", bufs=2) as attnp,
            tc.tile_pool(name="tmp", bufs=5) as tmpp,
            tc.tile_pool(name="tmp2", bufs=4) as tmp2p,
            tc.tile_pool(name="ebc", bufs=9) as ebcp,
            tc.tile_pool(name="outz", bufs=3) as outzp,
            tc.tile_pool(name="pmm", bufs=2, space="PSUM") as pmm,
            tc.tile_pool(name="pL", bufs=1, space="PSUM") as pLp,
        ):
            # ---- warm-up: keep PE busy during initial DMA so HAM un-throttles ----
            warm = consts.tile([P, 512], BF16, tag="warm")
            nc.vector.memset(warm, 0.0)
            for _ in range(8):
                ps = pmm.tile([P, HW], F32, tag="mm")
                nc.tensor.matmul(ps[:, :512], warm[:, :P], warm,
                                 start=True, stop=True)

            # ---- load constants ----
            # constants other than w1T/b1 go on the SWDGE queue so the sync
            # queue serves conv1's x/w chunks first (fast kernel start)
            def cload(name, dram, shape, dt=BF16, re="k p m -> p k m"):
                t = consts.tile(shape, dt, tag=name)
                nc.gpsimd.dma_start(out=t, in_=dram.rearrange(re) if re else dram)
                return t

            w1T = consts.tile([P, KC1, WIDTH], BF16, tag="w1T")
            b1 = consts.tile([P, PT, 1], F32, tag="b1")
            nc.gpsimd.dma_start(out=b1, in_=b1_d.rearrange("k p m -> p k m"))
            wqT = cload("wqT", wqT_d, [P, PT, WIDTH])
            wkT = cload("wkT", wkT_d, [P, PT, WIDTH])
            wvT = cload("wvT", wvT_d, [P, PT, WIDTH])
            w3T = cload("w3T", w3T_d, [P, PT, OUT])
            bq = cload("bq", bq_d, [P, PT, 1], F32)
            bk = cload("bk", bk_d, [P, PT, 1], F32)
            bv = cload("bv", bv_d, [P, PT, 1], F32)
            batt = cload("batt", batt_d, [P, PT, 1], F32)
            b3 = cload("b3", b3_d, [P, OC, 1], F32)
            sel = cload("sel", sel_d, [P, HEADS], re=None)
            p2 = cload("p2", p2_d, [P, PT, NT, P], re="k p m o -> p k m o")
            sab = cload("sab", sab_d, [P, HEADS], re=None)
            eye32 = cload("eye32", eye32_d, [HEADS, HEADS], re=None)
            ident = cload("ident", ident_d, [P, P], re=None)

            def rep4_bcast(q_eng, dst, src32):
                # dst[p, :] = src32[p // 4, :] — each row into 4 consecutive partitions
                bc = bass.AP(tensor=src32.tensor, offset=src32.offset,
                             ap=[list(src32.ap[0]), [0, 4]]
                                + [list(a) for a in src32.ap[1:]])
                q_eng.dma_start(out=dst, in_=bc)

            # persistent zero-padded k tiles, double-buffered per batch
            # (borders stay zero: only the interior is ever written)
            kpads = []
            for i in range(2):
                kp_ = consts.tile([P, PT, H + 2, W + 2], BF16, tag=f"kpad{i}")
                nc.vector.memset(kp_, 0.0)
                kpads.append(kp_)

            # v-side shifted-accumulation buffers: the spatial shift of each
            # of the 9 taps is applied as a free SOURCE OFFSET in the e
            # broadcast DMA and as a free RHS OFFSET in the accumulation
            # matmul, so the e*v product itself is a fully contiguous
            # unit-stride DVE op (2x bf16 mode). PAD=34 zero elements flank
            # the 1024-wide interiors so shifted reads stay in-bounds.
            PAD = 34
            WPAD = PAD + HW + PAD
            epk2s = []
            for i in range(2 * NT):
                ep_ = consts.tile([P, WPAD], BF16, tag=f"epk2_{i}")
                nc.vector.memset(ep_[:, :PAD], 0.0)
                nc.vector.memset(ep_[:, PAD + HW:], 0.0)
                epk2s.append(ep_)
            t2ps = []
            for i in range(3):
                t2_ = consts.tile([P, PT, WPAD], BF16, tag=f"t2p{i}")
                nc.vector.memset(t2_[:, :, :PAD], 0.0)
                nc.vector.memset(t2_[:, :, PAD + HW:], 0.0)
                t2ps.append(t2_)

            # per-batch state
            st = [dict() for _ in range(BL)]

            def load_x(b):
                xb = xbp.tile([P, KC1, HW], BF16, tag="xb")
                st[b]["xb"] = xb
                for kc in range(KC1):
                    if b == 0:
                        # separate HWDGE queue so w1T and x stream in parallel
                        nc.scalar.dma_start(out=w1T[:, kc, :], in_=w1T_d[kc])
                    nc.sync.dma_start(out=xb[:, kc, :], in_=x16_d[b, kc])

            def conv1(b):
                xb = st[b]["xb"]
                h1 = actp.tile([P, PT, HW], BF16, tag="h1")
                st[b]["h1"] = h1
                for mc in range(PT):
                    ps = pmm.tile([P, HW], F32, tag="mm")
                    for kc in range(KC1):
                        for n in range(NHALF):
                            nc.tensor.matmul(
                                ps[:, _ns(n)],
                                w1T[:, kc, mc * P:(mc + 1) * P],
                                xb[:, kc, _ns(n)],
                                start=(kc == 0), stop=(kc == KC1 - 1),
                            )
                    nc.scalar.activation(
                        out=h1[:, mc, :], in_=ps,
                        func=mybir.ActivationFunctionType.Relu,
                        bias=b1[:, mc], scale=1.0,
                    )

            def qkv_one(b, which):
                h1 = st[b]["h1"]
                kpad = kpads[b % 2]
                st[b]["kpad"] = kpad
                if which == "q":
                    q = actp.tile([P, PT, HW], BF16, tag="q")
                    st[b]["q"] = q
                elif which == "v":
                    v = actp.tile([P, PT, HW], BF16, tag="v")
                    st[b]["v"] = v
                wT, bias, relu = {
                    "q": (wqT, bq, True),
                    "k": (wkT, bk, True),
                    "v": (wvT, bv, False),
                }[which]
                for mc in range(PT):
                    ps = pmm.tile([P, HW], F32, tag="mm")
                    for kc in range(PT):
                        for n in range(NHALF):
                            nc.tensor.matmul(
                                ps[:, _ns(n)],
                                wT[:, kc, mc * P:(mc + 1) * P],
                                h1[:, kc, _ns(n)],
                                start=(kc == 0), stop=(kc == PT - 1),
                            )
                    if which == "k":
                        o = kpad[:, mc, 1:H + 1, 1:W + 1]
                        i = ps.rearrange("p (a b) -> p a b", a=H)
                    else:
                        o, i = st[b][which][:, mc, :], ps[:]
                    nc.scalar.activation(
                        out=o, in_=i,
                        func=(mybir.ActivationFunctionType.Relu if relu
                              else mybir.ActivationFunctionType.Identity),
                        bias=bias[:, mc], scale=1.0,
                    )

            def qkv(b):
                for which in ("q", "k", "v"):
                    qkv_one(b, which)

            def logits(b, filler=None):
                # packed tile t rows: 32*(kk%4) + g  for kk in {4t..4t+3}
                # filler: callbacks emitting independent PE work between
                # t-tiles so the PE queue never starves on DVE products
                q, kpad = st[b]["q"], st[b]["kpad"]
                epks = []
                st[b]["epks"] = epks
                den = attnp.tile([HEADS, HW], F32, tag="den")
                st[b]["den"] = den
                denp = pmm.tile([HEADS, HW], F32, tag="denp", bufs=1)
                for t in range(NT):
                    nsh = 4 if t < 2 else 1
                    rows = 32 * nsh
                    Lpk = pLp.tile([P, HW], F32, tag="Lpk")
                    st[b][f"Lpk{t}"] = Lpk
                    # qpos term: all rows at once per pt chunk
                    for n in range(NHALF):
                        for pt in range(PT):
                            nc.tensor.matmul(
                                Lpk[:rows, _ns(n)],
                                p2[:, pt, t, :rows],
                                q[:, pt, _ns(n)],
                                start=(pt == 0), stop=False,
                                skip_group_check=True,
                            )
                    # qk products (both chunks in one DVE op) + group reduce
                    for j in range(nsh):
                        kk = 4 * t + j
                        di, dj = kk // KS, kk % KS
                        tmp = tmpp.tile([P, PT, HW], BF16, tag="tmp")
                        nc.vector.tensor_tensor(
                            out=tmp.rearrange("p c (a b) -> p c a b", a=H),
                            in0=kpad[:, :, di:di + H, dj:dj + W],
                            in1=q.rearrange("p c (a b) -> p c a b", a=H),
                            op=mybir.AluOpType.mult,
                        )
                        for pt in range(PT):
                            for n in range(NHALF):
                                nc.tensor.matmul(
                                    Lpk[32 * j:32 * (j + 1), _ns(n)],
                                    sel,
                                    tmp[:, pt, _ns(n)],
                                    start=False, stop=(pt == PT - 1),
                                    tile_position=(0, 32 * j),
                                    skip_group_check=True,
                                )
                    epk = epk2s[NT * (b % 2) + t]
                    nc.scalar.activation(
                        out=epk[:rows, PAD:PAD + HW], in_=Lpk[:rows, :],
                        func=mybir.ActivationFunctionType.Exp,
                    )
                    epks.append(epk)
                    # issue this tile's e broadcasts now (sync/scalar HWDGE)
                    # so every eb has landed before the v phase needs it; the
                    # wrapped-column zeroing runs on the idle GpSimd engine so
                    # it never blocks the DVE product stream
                    for j in range(nsh):
                        kk = 4 * t + j
                        di, dj = kk // KS, kk % KS
                        dlt = 32 * (di - 1) + (dj - 1)
                        eb = ebcp.tile([P, HW], BF16, tag="ebc")
                        rep4_bcast(nc.sync if kk % 2 == 0 else nc.scalar,
                                   eb, epk[32 * j:32 * (j + 1),
                                           PAD - dlt:PAD - dlt + HW])
                        if dj == 0:
                            nc.gpsimd.memset(
                                eb.rearrange("p (a b) -> p a b", a=H)[:, :, W - 1],
                                0.0)
                        elif dj == 2:
                            nc.gpsimd.memset(
                                eb.rearrange("p (a b) -> p a b", a=H)[:, :, 0],
                                0.0)
                        st[b].setdefault("ebs", []).append(eb)
                    if filler is not None and t < len(filler):
                        filler[t]()
                # denominator accumulation (emitted after all exps so the PE
                # never waits on the ACT exp mid-phase)
                for t in range(NT):
                    rows = 128 if t < 2 else 32
                    lhs = sab if t < 2 else eye32
                    for n in range(NHALF):
                        nc.tensor.matmul(
                            denp[:, _ns(n)], lhs[:rows, :],
                            epks[t][:rows, PAD + n * 512:PAD + (n + 1) * 512],
                            start=(t == 0), stop=(t == NT - 1),
                            skip_group_check=True,
                        )
                nc.vector.reciprocal_approx_fast(out=den, in_=denp)

            def vphase(b):
                den, v, ebs = st[b]["den"], st[b]["v"], st[b]["ebs"]
                # recip broadcast head -> channels (same tile serves both chunks)
                recip_bc = attnp.tile([P, HW], F32, tag="recip_bc")
                rep4_bcast(nc.scalar, recip_bc, den)
                h2 = actp.tile([P, PT, HW], BF16, tag="h2")
                st[b]["h2"] = h2
                accs = [pmm.tile([P, HW], F32, tag="mm", name=f"acc{i}")
                        for i in range(PT)]
                for kk in range(NKK):
                    di, dj = kk // KS, kk % KS
                    dlt = 32 * (di - 1) + (dj - 1)
                    # fully contiguous product for both chunks (2x bf16 DVE):
                    # eb broadcast along the chunk dim via 0-stride
                    ebbc = bass.AP(
                        tensor=ebs[kk].tensor, offset=ebs[kk].offset,
                        ap=[list(ebs[kk].ap[0]), [0, PT], [1, HW]])
                    t2 = t2ps[kk % 3]
                    nc.vector.tensor_tensor(
                        out=t2[:, :, PAD:PAD + HW],
                        in0=ebbc,
                        in1=v,
                        op=mybir.AluOpType.mult,
                    )
                    for mc in range(PT):
                        for n in range(NHALF):
                            nc.tensor.matmul(
                                accs[mc][:, _ns(n)], ident,
                                t2[:, mc,
                                   PAD + dlt + n * 512:PAD + dlt + (n + 1) * 512],
                                start=(kk == 0), stop=(kk == NKK - 1),
                                skip_group_check=True,
                            )
                for mc in range(PT):
                    # h2 = relu(acc * recip_bc + batt)
                    t3 = tmp2p.tile([P, HW], F32, tag="t3", bufs=2)
                    nc.vector.tensor_tensor(
                        out=t3, in0=accs[mc], in1=recip_bc,
                        op=mybir.AluOpType.mult,
                    )
                    nc.scalar.activation(
                        out=h2[:, mc, :], in_=t3,
                        func=mybir.ActivationFunctionType.Relu,
                        bias=batt[:, mc], scale=1.0,
                    )

            def conv3_chunk(b, ocs, relu_split=False):
                # relu_split: drain odd output chunks on DVE instead of ACT —
                # used for the last batch where ACT is the tail pacer
                xb, h2 = st[b]["xb"], st[b]["h2"]
                for oc in ocs:
                    ps = pmm.tile([P, HW], F32, tag="mm")
                    for n in range(NHALF):
                        for kc in range(PT):
                            nc.tensor.matmul(
                                ps[:, _ns(n)],
                                w3T[:, kc, oc * P:(oc + 1) * P],
                                h2[:, kc, _ns(n)],
                                start=(kc == 0), stop=False,
                                skip_group_check=True,
                            )
                        nc.tensor.matmul(
                            ps[:, _ns(n)], ident, xb[:, oc, _ns(n)],
                            start=False, stop=True,
                            skip_group_check=True,
                        )
                    zr = outzp.tile([P, HW], BF16, tag="outzr")
                    if relu_split and oc % 2 == 1:
                        nc.vector.tensor_scalar(
                            out=zr, in0=ps, scalar1=b3[:, oc], scalar2=0.0,
                            op0=mybir.AluOpType.add, op1=mybir.AluOpType.max,
                        )
                    else:
                        nc.scalar.activation(
                            out=zr, in_=ps,
                            func=mybir.ActivationFunctionType.Relu,
                            bias=b3[:, oc], scale=1.0,
                        )
                    (nc.sync if oc % 2 == 0 else nc.scalar).dma_start(
                        out=out_d[b, oc], in_=zr)

            # ---- software-pipelined emission: PE always has independent
            # matmul work queued while the other batch's attention runs ----
            load_x(0)
            load_x(1)
            conv1(0)
            qkv(0)
            conv1(1)      # PE-filler while DVE makes b0 qk products
            logits(0, filler=[lambda: qkv_one(1, "q"),
                              lambda: qkv_one(1, "k"),
                              lambda: qkv_one(1, "v")])
            vphase(0)
            logits(1, filler=[lambda: conv3_chunk(0, (0, 1, 2)),
                              lambda: conv3_chunk(0, (3, 4, 5)),
                              lambda: conv3_chunk(0, (6, 7))])
            vphase(1)
            conv3_chunk(1, range(OC), relu_split=True)

    nc.compile()
    return nc


_PROG = None


def _perm():
    # channel permutation: c=g*8+d -> pt=d//4, p=g*4+(d%4)
    pm = np.zeros(WIDTH, np.int64)
    for c in range(WIDTH):
        g, d = c // D, c % D
        pm[(d // 4) * P + g * 4 + (d % 4)] = c
    return pm


def _host_prep(inputs):
    import ml_dtypes
    bf = ml_dtypes.bfloat16
    f = lambda a: np.asarray(a, dtype=np.float32)
    x = f(inputs["x"])
    pm = _perm()
    # fold bn scales into weights (bn(conv(x,W),s,b) = conv(x, s*W) + b)
    w1 = f(inputs["w_conv1"]) * f(inputs["bn1_s"])[:, None]
    wq = (f(inputs["wq"]) * f(inputs["bnq_s"])[:, None])[pm]
    wk = (f(inputs["wk"]) * f(inputs["bnk_s"])[:, None])[pm]
    # fold bnatt scale through the (linear) attention-value path into v
    sv = f(inputs["bnatt_s"]) * f(inputs["bnv_s"])
    wv = (f(inputs["wv"]) * sv[:, None])[pm]
    bv = (f(inputs["bnatt_s"]) * f(inputs["bnv_b"]))[pm]
    bq = f(inputs["bnq_b"])[pm]
    bk = f(inputs["bnk_b"])[pm]
    batt = f(inputs["bnatt_b"])[pm]
    w3 = (f(inputs["w_conv3"]) * f(inputs["bn3_s"])[:, None])[:, pm]

    posf = (f(inputs["pos_h"]) + f(inputs["pos_w"])).reshape(WIDTH, NKK)

    # sel[p, g] = 1 iff p // 4 == g  (same for both chunks in permuted layout)
    sel = np.zeros((P, HEADS), np.float32)
    for p in range(P):
        sel[p, p // 4] = 1.0
    # p2[pt, p, t, 32*j+g] = pos[c_global, 4t+j] if g == p // 4
    p2 = np.zeros((PT, P, NT, P), np.float32)
    for pt in range(PT):
        for p in range(P):
            g = p // 4
            c = pm[pt * P + p]
            for kk in range(NKK):
                t, j = kk // 4, kk % 4
                p2[pt, p, t, 32 * j + g] = posf[c, kk]
    # sab[r, g] = 1 if r % 32 == g (sum over the 4 packed kk rows)
    sab = np.zeros((P, HEADS), np.float32)
    for r in range(P):
        sab[r, r % HEADS] = 1.0
    com = {
        "w1T": np.ascontiguousarray(w1.T.reshape(KC1, P, WIDTH)).astype(bf),
        "wqT": np.ascontiguousarray(wq.T.reshape(PT, P, WIDTH)).astype(bf),
        "wkT": np.ascontiguousarray(wk.T.reshape(PT, P, WIDTH)).astype(bf),
        "wvT": np.ascontiguousarray(wv.T.reshape(PT, P, WIDTH)).astype(bf),
        "w3T": np.ascontiguousarray(w3.T.reshape(PT, P, OUT)).astype(bf),
        "b1": f(inputs["bn1_b"]).reshape(PT, P, 1),
        "bq": bq.reshape(PT, P, 1),
        "bk": bk.reshape(PT, P, 1),
        "bv": bv.reshape(PT, P, 1),
        "batt": batt.reshape(PT, P, 1),
        "b3": f(inputs["bn3_b"]).reshape(OC, P, 1),
        "sel": sel.astype(bf),
        "p2": p2.astype(bf),
        "sab": sab.astype(bf),
        "eye32": np.eye(HEADS, dtype=np.float32).astype(bf),
        "ident": np.eye(P, dtype=np.float32).astype(bf),
    }
    xr = x.reshape(B, KC1, P, HW)
    in_maps = []
    for c in range(NC_):
        xs = np.ascontiguousarray(xr[c * BL:(c + 1) * BL])
        in_maps.append(dict(com, x16=xs.astype(bf)))
    return in_maps


def kernel(**inputs):
    global _PROG
    if _PROG is None:
        _PROG = build_program()
    in_maps = _host_prep(inputs)
    res = run_bass_kernel_spmd(_PROG, in_maps, core_ids=list(range(NC_)))
    outs = [np.asarray(res.results[c]["out"], dtype=np.float32)
            .reshape(BL, OUT, H, W) for c in range(NC_)]
    return np.concatenate(outs, axis=0)
